# revision 39
# baseline (speedup 1.0000x reference)
"""Causal multi-head attention on 8 Trainium2 NeuronCores.

Problem: residual_stream [4, 2048, 1024] fp32, per-head QKV weights
[16, 1024, 64], output projection [1024, 1024]; causal softmax attention.

Sharding: (batch x head-group) — core c handles batch b = c//2 and head
group g = c%2 (heads 8g..8g+7), computes the partial output
y_partial[b] = concat_g @ W_out[512g:512(g+1)] and the host sums the two
group partials per batch.  Per-core HBM traffic is 8x lower than
head-only sharding: the input slice (4.2 MB bf16) is loaded once and
stays resident; the output partial is 8.4 MB.

Per-core pipeline (matmul operands bf16, accumulation fp32 in PSUM):
  1. For each of 4 head pairs: Q^T/K^T/V^T projections (stationary =
     weight m-tile, moving = X^T chunk), V^T -> V via PE transpose; V
     augmented with a ones column so P@V also emits softmax row sums.
  2. S^T = (Q K^T)^T blockwise, lower-triangle blocks only; the two
     heads of a pair run as concurrent K=64 row-group matmuls.
  3. P^T = exp(S^T/8) on ACT straight out of PSUM (scores are O(10), no
     max subtraction needed); diagonal blocks get a 0/1 mask on DVE.
     PV runs two k-tiles behind S^T/exp (software pipeline).
  4. O_aug^T = V_aug^T P^T accumulated per chunk; row 64 = row sums.
     Per chunk: sums staged to a 2-row tile, reciprocal on DVE, DRAM
     bounce broadcast, one gpsimd multiply normalizes the chunk.
  5. y[tok,:] = sum_p O_norm_p^T.T @ W_out[pair p rows] accumulated in
     PSUM over the 4 pairs; emitted per q-tile, streamed into pair 3's
     attention with a 2-chunk lag so the tail stays short.

Head-pair p+1's projections drip into pair p's attention to keep the PE
dense (HAM stays at full clock); per-chunk normalize thunks drip in with
them.
"""
import sys
import types
from collections import deque

sys.path.insert(0, "/opt/trn_rl_repo")

import ml_dtypes
import numpy as np

import concourse.bass as bass
import concourse.tile as tile
from concourse import mybir

F32 = mybir.dt.float32
F32R = mybir.dt.float32r
BF16 = mybir.dt.bfloat16

B = 4
SEQ = 2048
DM = 1024
DH = 64
NH = 16
NCORES = 8
NPAIR = 4                   # head pairs per core (8 heads)
MT = DM // 128              # m-tiles = 8
KT = SEQ // 128             # k-tiles = 16
QC = SEQ // 512             # q-chunks of 512 = 4

_CACHE = {}


def _split_waits(d, max_waits=1):
    # This walrus build allows a single sync-wait on several instruction
    # encodings (CTRL Drain, fused-LDW f32 Matmult). Hoist excess waits
    # onto same-engine NoOp carriers directly in the BIR JSON.
    for fn in d.get("functions", []):
        for blk in fn.get("blocks", []):
            out = []
            for inst in blk.get("instructions", []):
                si = inst.get("sync_info") or {}
                waits = si.get("on_wait") or []
                if len(waits) > max_waits:
                    extra = waits[: len(waits) - max_waits]
                    rest = waits[len(waits) - max_waits:]
                    for i, w in enumerate(extra):
                        out.append({
                            "name": f"{inst['name']}_sw{i}",
                            "opcode": "NoOp",
                            "engine": inst["engine"],
                            "ins": [],
                            "outs": [],
                            "sync_info": {"on_update": [], "on_wait": [w]},
                        })
                    inst = dict(inst)
                    inst["sync_info"] = {
                        "on_update": list(si.get("on_update") or []),
                        "on_wait": rest,
                    }
                out.append(inst)
            blk["instructions"] = out
    return d


def _patch_nc(nc):
    import orjson

    def to_json_bytes(self):
        return orjson.dumps(
            _split_waits(orjson.loads(mybir.module_to_json_bytes(self.m)))
        )

    nc.to_json_bytes = types.MethodType(to_json_bytes, nc)
    return nc


def _build_nc():
    nc = bass.Bass()

    # xt[p, ((cq*MT + m)*512 + q)] = residual[b, cq*512 + q, m*128 + p]
    xt = nc.dram_tensor("xt", [128, QC, MT, 512], BF16, kind="ExternalInput")
    # w_all[kp, m, t, p*128 + hh*64 + d] = W_t[8g + 2p + hh, 128m + kp, d]
    w_all = nc.dram_tensor("w_all", [128, MT, 3, NPAIR * 128], BF16,
                           kind="ExternalInput")
    # wout[kp, p, n] = weight_out[512g + 128p + kp, n]
    wout = nc.dram_tensor("wout", [128, NPAIR, DM], F32R,
                          kind="ExternalInput")
    masks = nc.dram_tensor("masks", [128, 4, 512], BF16, kind="ExternalInput")
    ident = nc.dram_tensor("ident", [128, 128], BF16, kind="ExternalInput")
    ones_bf = nc.dram_tensor("ones_bf", [1, 64], BF16, kind="ExternalInput")
    y = nc.dram_tensor("y", [SEQ, DM], F32, kind="ExternalOutput")
    rscr = nc.dram_tensor("rscr", [NPAIR, 2 * QC, 512], F32)  # recip bounce

    with tile.TileContext(nc) as tc:
        with (
            tc.tile_pool(name="const", bufs=1) as const,
            tc.tile_pool(name="qkp", bufs=2) as qkp,
            tc.tile_pool(name="vtp", bufs=1) as vtp,
            tc.tile_pool(name="ptp", bufs=4) as ptp,
            tc.tile_pool(name="onp", bufs=4) as onp,
            tc.tile_pool(name="stg", bufs=4) as stg,
            tc.tile_pool(name="small", bufs=2) as small,
            tc.tile_pool(name="bcp", bufs=2) as bcp,
            tc.tile_pool(name="yp", bufs=2) as yp,
            tc.tile_pool(name="pss", bufs=2, space="PSUM") as pss,
            tc.tile_pool(name="pso", bufs=2, space="PSUM") as pso,
            tc.tile_pool(name="psf", bufs=2, space="PSUM") as psf,
        ):
            # ---- preload: two HWDGE queues in parallel -------------------
            w_t = const.tile([128, MT, 3, NPAIR * 128], BF16, tag="w")
            for m0 in range(0, MT, 2):
                nc.scalar.dma_start(out=w_t[:, m0:m0 + 2],
                                    in_=w_all[:, m0:m0 + 2])

            xt_t = const.tile([128, QC, MT, 512], BF16, tag="xt")
            for cq in range(QC):
                nc.sync.dma_start(out=xt_t[:, cq], in_=xt[:, cq])
            mask_t = const.tile([128, 4, 512], BF16, tag="mask")
            nc.sync.dma_start(out=mask_t, in_=masks[:])
            ident_t = const.tile([128, 128], BF16, tag="ident")
            nc.sync.dma_start(out=ident_t, in_=ident[:])
            wout_t = const.tile([128, NPAIR, DM], F32R, tag="wout")
            nc.sync.dma_start(out=wout_t, in_=wout[:])

            # V_aug double-buffered (pair parity); ones columns filled once
            # by memset (a broadcast DMA here floods the SDMA engines with
            # 2-byte descriptors and stalls every other queue for ~80us).
            vaug0 = const.tile([128, KT, 2, 65], BF16, tag="vaug0")
            vaug1 = const.tile([128, KT, 2, 65], BF16, tag="vaug1")
            vaugs = [vaug0, vaug1]
            nc.gpsimd.memset(vaug0[:, :, :, 64:65], 1.0)
            nc.gpsimd.memset(vaug1[:, :, :, 64:65], 1.0)

            def gen_proj(p, out):
                """QKV projection thunks for head pair p (reads resident
                xt_t; one thunk per (chunk, m-tile) step)."""
                ps = slice(p * 128, (p + 1) * 128)
                qt = qkp.tile([128, SEQ], BF16, tag="qt", name=f"qt_{p}")
                kt = qkp.tile([128, SEQ], BF16, tag="kt", name=f"kt_{p}")
                vt = vtp.tile([128, SEQ], BF16, tag="vt", name=f"vt_{p}")
                out["qt"], out["kt"], out["vt"] = qt, kt, vt
                thunks = []
                state = {}

                def qkstep(cq, m):
                    cqs = slice(cq * 512, cq * 512 + 512)
                    if m == 0:
                        state["pq"] = psf.tile([128, 512], F32, tag="bank",
                                               name=f"pq_{p}c{cq}")
                        state["pk"] = psf.tile([128, 512], F32, tag="bank",
                                               name=f"pk_{p}c{cq}")
                    for proj, key in ((0, "pq"), (1, "pk")):
                        nc.tensor.matmul(
                            state[key][:],
                            w_t[:, m, proj, ps],
                            xt_t[:, cq, m, :],
                            start=(m == 0),
                            stop=(m == MT - 1),
                        )
                    if m == MT - 1:
                        nc.scalar.copy(qt[:, cqs], state["pq"][:])
                        nc.vector.tensor_copy(kt[:, cqs], state["pk"][:])

                def vstep(cq, m):
                    cqs = slice(cq * 512, cq * 512 + 512)
                    if m == 0:
                        state["pv"] = psf.tile([128, 512], F32, tag="bank",
                                               name=f"pv_{p}c{cq}")
                    nc.tensor.matmul(
                        state["pv"][:],
                        w_t[:, m, 2, ps],
                        xt_t[:, cq, m, :],
                        start=(m == 0),
                        stop=(m == MT - 1),
                    )
                    if m == MT - 1:
                        nc.vector.tensor_copy(vt[:, cqs], state["pv"][:])

                for cq in range(QC):
                    for m in range(MT):
                        thunks.append(lambda cq=cq, m=m: qkstep(cq, m))
                    for m in range(MT):
                        thunks.append(lambda cq=cq, m=m: vstep(cq, m))
                return thunks

            def gen_vtrans(p, vt, vaug):
                """V^T -> V_aug transposes as PE thunks; one DVE copy per
                k-tile fills both heads' columns."""
                def tstep(tk):
                    pt_ps = psf.tile([128, 128], BF16, tag="bank",
                                     name=f"tp_{p}t{tk}")
                    nc.tensor.transpose(
                        pt_ps[:], vt[:, tk * 128:(tk + 1) * 128], ident_t[:]
                    )
                    nc.vector.tensor_copy(vaug[:, tk, :, 0:64], pt_ps[:])
                return [lambda tk=tk: tstep(tk) for tk in range(KT)]

            def mask_bcast(j):
                t = mask_t[:, j, :]
                return bass.AP(
                    tensor=t.tensor, offset=t.offset,
                    ap=[list(t.ap[0]), [0, 2], [1, 512]],
                )

            def gen_attention_chunks(p, qt, kt, onorm, vaug):
                """Attention thunks for pair p, returned per chunk. The two
                heads' S^T blocks land in one [128,1024] PSUM pair and are
                exponentiated by a single ACT op."""
                chunk_lists = []
                for cq in range(QC):
                    ntk = 4 * cq + 4
                    st = {"ops": None, "pend": [], "stage": None}

                    def make_pt(cq, tk):
                        cqs = slice(cq * 512, cq * 512 + 512)
                        tks = slice(tk * 128, tk * 128 + 128)
                        sdbl = pss.tile([128, 1024], F32, tag="bank",
                                        name=f"s_p{p}c{cq}t{tk}")
                        for h in (0, 1):
                            hs = slice(h * 64, h * 64 + 64)
                            nc.tensor.matmul(
                                sdbl[:, h * 512:(h + 1) * 512],
                                kt[hs, tks], qt[hs, cqs],
                                start=True, stop=True,
                            )
                        pt = ptp.tile([128, 1024], BF16, tag="pt",
                                      name=f"pt_p{p}c{cq}t{tk}")
                        nc.scalar.activation(
                            pt[:], sdbl[:],
                            mybir.ActivationFunctionType.Exp,
                            bias=0.0, scale=0.125,
                        )
                        if tk >= 4 * cq:
                            nc.vector.tensor_mul(
                                pt[:], pt[:], mask_bcast(tk - 4 * cq)
                            )
                        return pt

                    def pv_step(cq, tk, pt, st, ntk):
                        for h in (0, 1):
                            nc.tensor.matmul(
                                st["ops"][h][:], vaug[:, tk, h, :],
                                pt[:, h * 512:(h + 1) * 512],
                                start=(tk == 0), stop=(tk == ntk - 1),
                            )

                    def finish_chunk(cq, st):
                        cqs = slice(cq * 512, cq * 512 + 512)
                        stage = stg.tile([2, 512], F32, tag="stage",
                                         name=f"stage_p{p}c{cq}")
                        st["stage"] = stage
                        for h in (0, 1):
                            scr64 = small.tile([65, 512], F32, tag="scr64",
                                               name=f"scr64_p{p}c{cq}h{h}")
                            nc.vector.tensor_copy(scr64[64:65, :],
                                                  st["ops"][h][64:65, :])
                            nc.sync.dma_start(
                                out=stage[h:h + 1, :],
                                in_=scr64[64:65, :],
                            )
                            nc.vector.tensor_copy(
                                onorm[64 * h:64 * h + 64, cqs],
                                st["ops"][h][0:64, :]
                            )

                    def step(cq, tk, st, ntk):
                        if tk == 0:
                            st["ops"] = {
                                h: pso.tile([65, 512], F32, tag="bank",
                                            name=f"ops_p{p}c{cq}h{h}")
                                for h in (0, 1)
                            }
                        st["pend"].append((tk, make_pt(cq, tk)))
                        if len(st["pend"]) > 3:
                            t0, p0 = st["pend"].pop(0)
                            pv_step(cq, t0, p0, st, ntk)
                        if tk == ntk - 1:
                            while st["pend"]:
                                t0, p0 = st["pend"].pop(0)
                                pv_step(cq, t0, p0, st, ntk)
                            finish_chunk(cq, st)

                    chunk_lists.append(
                        ([lambda cq=cq, tk=tk, st=st, ntk=ntk:
                          step(cq, tk, st, ntk) for tk in range(ntk)], st)
                    )
                return chunk_lists

            def gen_norm_chunk(p, cq, onorm, st):
                """Per-chunk normalize: reciprocal of the chunk's two sums
                rows, DRAM-bounce broadcast (SBUF-source broadcast DMAs
                serialize on one SBUF port, ~12us), one gpsimd mul."""
                def norm():
                    cqs = slice(cq * 512, cq * 512 + 512)
                    recip = stg.tile([2, 512], F32, tag="recip",
                                     name=f"recip_p{p}c{cq}")
                    nc.vector.reciprocal(recip[:], st["stage"][:])
                    nc.sync.dma_start(out=rscr[p, 2 * cq:2 * cq + 2],
                                      in_=recip[:])
                    bc = bcp.tile([128, 512], F32, tag="bcc",
                                  name=f"bc_p{p}c{cq}")
                    for h in (0, 1):
                        bc_src = bass.AP(
                            tensor=rscr[:].tensor,
                            offset=(p * 2 * QC + 2 * cq + h) * 512,
                            ap=[[0, 64], [1, 512]],
                        )
                        nc.gpsimd.dma_start(out=bc[64 * h:64 * h + 64, :],
                                            in_=bc_src)
                    nc.gpsimd.tensor_mul(
                        onorm[:, cqs], onorm.bitcast(F32)[:, cqs], bc[:]
                    )
                return [norm]

            def gen_wout(cq, onorms):
                """Output projection for q-tiles of chunk cq, accumulating
                all 4 pairs' onorm in PSUM."""
                def wstep(qi):
                    ysb = yp.tile([128, DM], F32, tag="y", name=f"y_{qi}")
                    for nh in range(2):
                        yps = psf.tile([128, 512], F32, tag="bank",
                                       name=f"yps_{qi}_{nh}")
                        for p in range(NPAIR):
                            nc.tensor.matmul(
                                yps[:],
                                onorms[p][:, qi * 128:(qi + 1) * 128],
                                wout_t[:, p, nh * 512:(nh + 1) * 512],
                                start=(p == 0), stop=(p == NPAIR - 1),
                            )
                        if nh == 0:
                            nc.scalar.copy(ysb[:, 0:512], yps[:])
                        else:
                            nc.vector.tensor_copy(ysb[:, 512:1024], yps[:])
                    nc.sync.dma_start(
                        out=y[qi * 128:(qi + 1) * 128, :], in_=ysb
                    )
                return [lambda qi=qi: wstep(qi)
                        for qi in range(4 * cq, 4 * cq + 4)]

            def drive_chunk(steps, fq, quota):
                """Run chunk steps, dripping up to `quota` thunks from fq
                evenly between steps."""
                n = len(steps)
                fi = 0
                for i, t in enumerate(steps):
                    t()
                    want = min(quota, len(fq) + fi) * (i + 1) // n
                    while fi < want and fq:
                        fq.popleft()()
                        fi += 1

            # ---- main schedule ------------------------------------------
            onorms = []
            cur = {}
            fq = deque()
            for t in gen_proj(0, cur):
                t()
            # k-tiles 0-3 of V_aug are needed by attention chunk 0
            # immediately; each chunk then emits the NEXT chunk's four
            # transposes as a mandatory prefix (PE filler that is
            # guaranteed to precede the PV matmuls that read the tiles —
            # leaving them in the drip queue can emit them after).
            vtr = gen_vtrans(0, cur["vt"], vaugs[0])
            for t in vtr[:4]:
                t()
            vtr_rest = vtr[4:]

            for p in range(NPAIR):
                onorm = onp.tile([128, SEQ], F32R, tag="onorm",
                                 name=f"onorm_{p}")
                onorms.append(onorm)
                chunks = gen_attention_chunks(p, cur["qt"], cur["kt"],
                                              onorm, vaugs[p % 2])
                nxt = {}
                if p + 1 < NPAIR:
                    fq.extend(gen_proj(p + 1, nxt))
                total = sum(len(s) for s, _ in chunks)
                done = 0
                for cq, (steps, st) in enumerate(chunks):
                    for t in vtr_rest[4 * cq:4 * cq + 4]:
                        t()
                    if p == NPAIR - 1 and cq >= 2:
                        fq.extend(gen_wout(cq - 2, onorms))
                    rem = total - done
                    quota = (len(fq) * len(steps) + rem - 1) // rem
                    drive_chunk(steps, fq, quota)
                    done += len(steps)
                    if cq < QC - 1:
                        fq.extendleft(
                            reversed(gen_norm_chunk(p, cq, onorm, st)))
                # flush before the next pair's transposes (the leftover
                # projection steps write vt, which vtrans reads).
                while fq:
                    fq.popleft()()
                if p + 1 < NPAIR:
                    vtr = gen_vtrans(p + 1, nxt["vt"], vaugs[(p + 1) % 2])
                    for t in vtr[:4]:
                        t()
                    vtr_rest = vtr[4:]
                else:
                    vtr_rest = []
                # the boundary chunk's normalize goes to the back of the
                # queue so its 3.3us DVE reciprocal lands behind the next
                # pair's first diagonal mask-muls and V_aug copies.
                fq.extend(gen_norm_chunk(p, QC - 1, onorm, chunks[QC - 1][1]))
                cur = nxt
            # tail: last chunk's normalize first, then wout(2) overlaps the
            # normalize chain's latency before the gated wout(3) runs.
            while fq:
                fq.popleft()()
            for t in gen_wout(2, onorms):
                t()
            for t in gen_wout(3, onorms):
                t()

    return _patch_nc(nc)


def _causal_masks():
    # masks[kp, j, f] = f >= kp + 128*j  (block-diagonal causal masks)
    m = np.zeros((128, 4, 512), np.float32)
    i = np.arange(128)[:, None]
    f = np.arange(512)[None, :]
    for j in range(4):
        m[:, j, :] = (f >= i + 128 * j).astype(np.float32)
    return m


def _prepare_in_maps(residual_stream, weight_query, weight_key, weight_value,
                     weight_out):
    bf16 = ml_dtypes.bfloat16
    r = np.asarray(residual_stream, np.float32)
    xts = []
    for b in range(B):
        xtb = r[b].T.reshape(MT, 128, QC, 512)          # [m, p, cq, q]
        xtb = xtb.transpose(1, 2, 0, 3)                 # [p, cq, m, q]
        xts.append(np.ascontiguousarray(xtb.astype(bf16)))
    masks = _causal_masks().astype(bf16)
    ident = np.eye(128, dtype=np.float32).astype(bf16)
    ones = np.ones((1, 64), bf16)
    wo_f = np.asarray(weight_out, np.float32)
    in_maps = []
    for c in range(NCORES):
        b, g = c // 2, c % 2
        w = np.empty((128, MT, 3, NPAIR * 128), np.float32)
        for t, wt in enumerate((weight_query, weight_key, weight_value)):
            wg = np.asarray(wt, np.float32)[8 * g:8 * g + 8]
            wg = wg.reshape(NPAIR, 2, MT, 128, DH)      # [p, hh, m, kp, d]
            w[:, :, t, :] = wg.transpose(3, 2, 0, 1, 4).reshape(
                128, MT, NPAIR * 128)
        wo = wo_f[512 * g:512 * (g + 1)].reshape(NPAIR, 128, DM)
        wo = np.ascontiguousarray(wo.transpose(1, 0, 2))
        in_maps.append({
            "xt": xts[b],
            "w_all": np.ascontiguousarray(w.astype(bf16)),
            "wout": wo,
            "masks": masks,
            "ident": ident,
            "ones_bf": ones,
        })
    return in_maps


def kernel(residual_stream, weight_query, weight_key, weight_value,
           weight_out, trace=False):
    from concourse.bass_utils import run_bass_kernel_spmd

    if "nc" not in _CACHE:
        _CACHE["nc"] = _build_nc()
    nc = _CACHE["nc"]

    in_maps = _prepare_in_maps(
        residual_stream, weight_query, weight_key, weight_value, weight_out
    )
    res = run_bass_kernel_spmd(
        nc, in_maps, list(range(NCORES)), trace=trace
    )
    _CACHE["last_result"] = res
    out = np.empty((B, SEQ, DM), np.float32)
    for b in range(B):
        out[b] = res.results[2 * b]["y"] + res.results[2 * b + 1]["y"]
    return out


# revision 40
# speedup vs baseline: 1.0194x; 1.0194x over previous
"""Causal multi-head attention on 8 Trainium2 NeuronCores.

Problem: residual_stream [4, 2048, 1024] fp32, per-head QKV weights
[16, 1024, 64], output projection [1024, 1024]; causal softmax attention.

Sharding: (batch x head-group) — core c handles batch b = c//2 and head
group g = c%2 (heads 8g..8g+7), computes the partial output
y_partial[b] = concat_g @ W_out[512g:512(g+1)] and the host sums the two
group partials per batch.  Per-core HBM traffic is 8x lower than
head-only sharding: the input slice (4.2 MB bf16) is loaded once and
stays resident; the output partial is 8.4 MB.

Per-core pipeline (matmul operands bf16, accumulation fp32 in PSUM):
  1. For each of 4 head pairs: Q^T/K^T/V^T projections (stationary =
     weight m-tile, moving = X^T chunk), V^T -> V via PE transpose; V
     augmented with a ones column so P@V also emits softmax row sums.
  2. S^T = (Q K^T)^T blockwise, lower-triangle blocks only; the two
     heads of a pair run as concurrent K=64 row-group matmuls.
  3. P^T = exp(S^T/8) on ACT straight out of PSUM (scores are O(10), no
     max subtraction needed); diagonal blocks get a 0/1 mask on DVE.
     PV runs two k-tiles behind S^T/exp (software pipeline).
  4. O_aug^T = V_aug^T P^T accumulated per chunk; row 64 = row sums.
     Per chunk: sums staged to a 2-row tile, reciprocal on DVE, DRAM
     bounce broadcast, one gpsimd multiply normalizes the chunk.
  5. y[tok,:] = sum_p O_norm_p^T.T @ W_out[pair p rows] accumulated in
     PSUM over the 4 pairs; emitted per q-tile, streamed into pair 3's
     attention with a 2-chunk lag so the tail stays short.

Head-pair p+1's projections drip into pair p's attention to keep the PE
dense (HAM stays at full clock); per-chunk normalize thunks drip in with
them.
"""
import sys
import types
from collections import deque

sys.path.insert(0, "/opt/trn_rl_repo")

import ml_dtypes
import numpy as np

import concourse.bass as bass
import concourse.tile as tile
from concourse import mybir

F32 = mybir.dt.float32
F32R = mybir.dt.float32r
BF16 = mybir.dt.bfloat16

B = 4
SEQ = 2048
DM = 1024
DH = 64
NH = 16
NCORES = 8
NPAIR = 4                   # head pairs per core (8 heads)
MT = DM // 128              # m-tiles = 8
KT = SEQ // 128             # k-tiles = 16
QC = SEQ // 512             # q-chunks of 512 = 4

_CACHE = {}


def _split_waits(d, max_waits=1):
    # This walrus build allows a single sync-wait on several instruction
    # encodings (CTRL Drain, fused-LDW f32 Matmult). Hoist excess waits
    # onto same-engine NoOp carriers directly in the BIR JSON.
    for fn in d.get("functions", []):
        for blk in fn.get("blocks", []):
            out = []
            for inst in blk.get("instructions", []):
                si = inst.get("sync_info") or {}
                waits = si.get("on_wait") or []
                if len(waits) > max_waits:
                    extra = waits[: len(waits) - max_waits]
                    rest = waits[len(waits) - max_waits:]
                    for i, w in enumerate(extra):
                        out.append({
                            "name": f"{inst['name']}_sw{i}",
                            "opcode": "NoOp",
                            "engine": inst["engine"],
                            "ins": [],
                            "outs": [],
                            "sync_info": {"on_update": [], "on_wait": [w]},
                        })
                    inst = dict(inst)
                    inst["sync_info"] = {
                        "on_update": list(si.get("on_update") or []),
                        "on_wait": rest,
                    }
                out.append(inst)
            blk["instructions"] = out
    return d


def _patch_nc(nc):
    import orjson

    def to_json_bytes(self):
        return orjson.dumps(
            _split_waits(orjson.loads(mybir.module_to_json_bytes(self.m)))
        )

    nc.to_json_bytes = types.MethodType(to_json_bytes, nc)
    return nc


def _build_nc():
    nc = bass.Bass()

    # xt[p, ((cq*MT + m)*512 + q)] = residual[b, cq*512 + q, m*128 + p]
    xt = nc.dram_tensor("xt", [128, QC, MT, 512], BF16, kind="ExternalInput")
    # w_all[kp, m, t, p*128 + hh*64 + d] = W_t[8g + 2p + hh, 128m + kp, d]
    w_all = nc.dram_tensor("w_all", [128, MT, 3, NPAIR * 128], BF16,
                           kind="ExternalInput")
    # wout[kp, p, n] = weight_out[512g + 128p + kp, n]
    wout = nc.dram_tensor("wout", [128, NPAIR, DM], F32R,
                          kind="ExternalInput")
    masks = nc.dram_tensor("masks", [128, 4, 512], BF16, kind="ExternalInput")
    ident = nc.dram_tensor("ident", [128, 128], BF16, kind="ExternalInput")
    ones_bf = nc.dram_tensor("ones_bf", [1, 64], BF16, kind="ExternalInput")
    y = nc.dram_tensor("y", [SEQ, DM], F32, kind="ExternalOutput")
    rscr = nc.dram_tensor("rscr", [NPAIR, 2 * QC, 512], F32)  # recip bounce

    with tile.TileContext(nc) as tc:
        with (
            tc.tile_pool(name="const", bufs=1) as const,
            tc.tile_pool(name="qkp", bufs=2) as qkp,
            tc.tile_pool(name="vtp", bufs=1) as vtp,
            tc.tile_pool(name="ptp", bufs=4) as ptp,
            tc.tile_pool(name="onp", bufs=4) as onp,
            tc.tile_pool(name="stg", bufs=4) as stg,
            tc.tile_pool(name="small", bufs=2) as small,
            tc.tile_pool(name="bcp", bufs=2) as bcp,
            tc.tile_pool(name="yp", bufs=2) as yp,
            tc.tile_pool(name="pss", bufs=2, space="PSUM") as pss,
            tc.tile_pool(name="pso", bufs=2, space="PSUM") as pso,
            tc.tile_pool(name="psf", bufs=2, space="PSUM") as psf,
        ):
            # ---- preload: two HWDGE queues in parallel -------------------
            w_t = const.tile([128, MT, 3, NPAIR * 128], BF16, tag="w")
            nc.scalar.dma_start(out=w_t[:, 0:4], in_=w_all[:, 0:4])
            nc.scalar.dma_start(out=w_t[:, 4:8], in_=w_all[:, 4:8])

            xt_t = const.tile([128, QC, MT, 512], BF16, tag="xt")
            for cq in range(QC):
                nc.sync.dma_start(out=xt_t[:, cq], in_=xt[:, cq])
            mask_t = const.tile([128, 4, 512], BF16, tag="mask")
            nc.sync.dma_start(out=mask_t, in_=masks[:])
            ident_t = const.tile([128, 128], BF16, tag="ident")
            nc.sync.dma_start(out=ident_t, in_=ident[:])
            wout_t = const.tile([128, NPAIR, DM], F32R, tag="wout")
            nc.sync.dma_start(out=wout_t, in_=wout[:])

            # V_aug double-buffered (pair parity); ones columns filled once
            # by memset (a broadcast DMA here floods the SDMA engines with
            # 2-byte descriptors and stalls every other queue for ~80us).
            vaug0 = const.tile([128, KT, 2, 65], BF16, tag="vaug0")
            vaug1 = const.tile([128, KT, 2, 65], BF16, tag="vaug1")
            vaugs = [vaug0, vaug1]
            nc.gpsimd.memset(vaug0[:, :, :, 64:65], 1.0)
            nc.gpsimd.memset(vaug1[:, :, :, 64:65], 1.0)

            def gen_proj(p, out):
                """QKV projection thunks for head pair p (reads resident
                xt_t; one thunk per (chunk, m-tile) step)."""
                ps = slice(p * 128, (p + 1) * 128)
                qt = qkp.tile([128, SEQ], BF16, tag="qt", name=f"qt_{p}")
                kt = qkp.tile([128, SEQ], BF16, tag="kt", name=f"kt_{p}")
                vt = vtp.tile([128, SEQ], BF16, tag="vt", name=f"vt_{p}")
                out["qt"], out["kt"], out["vt"] = qt, kt, vt
                thunks = []
                state = {}

                def qkstep(cq, m):
                    cqs = slice(cq * 512, cq * 512 + 512)
                    if m == 0:
                        state["pq"] = psf.tile([128, 512], F32, tag="bank",
                                               name=f"pq_{p}c{cq}")
                        state["pk"] = psf.tile([128, 512], F32, tag="bank",
                                               name=f"pk_{p}c{cq}")
                    for proj, key in ((0, "pq"), (1, "pk")):
                        nc.tensor.matmul(
                            state[key][:],
                            w_t[:, m, proj, ps],
                            xt_t[:, cq, m, :],
                            start=(m == 0),
                            stop=(m == MT - 1),
                        )
                    if m == MT - 1:
                        nc.scalar.copy(qt[:, cqs], state["pq"][:])
                        nc.vector.tensor_copy(kt[:, cqs], state["pk"][:])

                def vstep(cq, m):
                    cqs = slice(cq * 512, cq * 512 + 512)
                    if m == 0:
                        state["pv"] = psf.tile([128, 512], F32, tag="bank",
                                               name=f"pv_{p}c{cq}")
                    nc.tensor.matmul(
                        state["pv"][:],
                        w_t[:, m, 2, ps],
                        xt_t[:, cq, m, :],
                        start=(m == 0),
                        stop=(m == MT - 1),
                    )
                    if m == MT - 1:
                        nc.vector.tensor_copy(vt[:, cqs], state["pv"][:])

                for cq in range(QC):
                    for m in range(MT):
                        thunks.append(lambda cq=cq, m=m: qkstep(cq, m))
                    for m in range(MT):
                        thunks.append(lambda cq=cq, m=m: vstep(cq, m))
                return thunks

            def gen_vtrans(p, vt, vaug):
                """V^T -> V_aug transposes as PE thunks; one DVE copy per
                k-tile fills both heads' columns."""
                def tstep(tk):
                    pt_ps = psf.tile([128, 128], BF16, tag="bank",
                                     name=f"tp_{p}t{tk}")
                    nc.tensor.transpose(
                        pt_ps[:], vt[:, tk * 128:(tk + 1) * 128], ident_t[:]
                    )
                    nc.vector.tensor_copy(vaug[:, tk, :, 0:64], pt_ps[:])
                return [lambda tk=tk: tstep(tk) for tk in range(KT)]

            def mask_bcast(j):
                t = mask_t[:, j, :]
                return bass.AP(
                    tensor=t.tensor, offset=t.offset,
                    ap=[list(t.ap[0]), [0, 2], [1, 512]],
                )

            def gen_attention_chunks(p, qt, kt, onorm, vaug):
                """Attention thunks for pair p, returned per chunk. The two
                heads' S^T blocks land in one [128,1024] PSUM pair and are
                exponentiated by a single ACT op."""
                chunk_lists = []
                for cq in range(QC):
                    ntk = 4 * cq + 4
                    st = {"ops": None, "pend": [], "stage": None}

                    def make_pt(cq, tk):
                        cqs = slice(cq * 512, cq * 512 + 512)
                        tks = slice(tk * 128, tk * 128 + 128)
                        sdbl = pss.tile([128, 1024], F32, tag="bank",
                                        name=f"s_p{p}c{cq}t{tk}")
                        for h in (0, 1):
                            hs = slice(h * 64, h * 64 + 64)
                            nc.tensor.matmul(
                                sdbl[:, h * 512:(h + 1) * 512],
                                kt[hs, tks], qt[hs, cqs],
                                start=True, stop=True,
                            )
                        pt = ptp.tile([128, 1024], BF16, tag="pt",
                                      name=f"pt_p{p}c{cq}t{tk}")
                        nc.scalar.activation(
                            pt[:], sdbl[:],
                            mybir.ActivationFunctionType.Exp,
                            bias=0.0, scale=0.125,
                        )
                        if tk >= 4 * cq:
                            nc.vector.tensor_mul(
                                pt[:], pt[:], mask_bcast(tk - 4 * cq)
                            )
                        return pt

                    def pv_step(cq, tk, pt, st, ntk):
                        for h in (0, 1):
                            nc.tensor.matmul(
                                st["ops"][h][:], vaug[:, tk, h, :],
                                pt[:, h * 512:(h + 1) * 512],
                                start=(tk == 0), stop=(tk == ntk - 1),
                            )

                    def finish_chunk(cq, st):
                        cqs = slice(cq * 512, cq * 512 + 512)
                        stage = stg.tile([2, 512], F32, tag="stage",
                                         name=f"stage_p{p}c{cq}")
                        st["stage"] = stage
                        for h in (0, 1):
                            scr64 = small.tile([65, 512], F32, tag="scr64",
                                               name=f"scr64_p{p}c{cq}h{h}")
                            nc.vector.tensor_copy(scr64[64:65, :],
                                                  st["ops"][h][64:65, :])
                            nc.sync.dma_start(
                                out=stage[h:h + 1, :],
                                in_=scr64[64:65, :],
                            )
                            nc.vector.tensor_copy(
                                onorm[64 * h:64 * h + 64, cqs],
                                st["ops"][h][0:64, :]
                            )

                    def step(cq, tk, st, ntk):
                        if tk == 0:
                            st["ops"] = {
                                h: pso.tile([65, 512], F32, tag="bank",
                                            name=f"ops_p{p}c{cq}h{h}")
                                for h in (0, 1)
                            }
                        st["pend"].append((tk, make_pt(cq, tk)))
                        if len(st["pend"]) > 3:
                            t0, p0 = st["pend"].pop(0)
                            pv_step(cq, t0, p0, st, ntk)
                        if tk == ntk - 1:
                            while st["pend"]:
                                t0, p0 = st["pend"].pop(0)
                                pv_step(cq, t0, p0, st, ntk)
                            finish_chunk(cq, st)

                    chunk_lists.append(
                        ([lambda cq=cq, tk=tk, st=st, ntk=ntk:
                          step(cq, tk, st, ntk) for tk in range(ntk)], st)
                    )
                return chunk_lists

            def gen_norm_chunk(p, cq, onorm, st):
                """Per-chunk normalize: reciprocal of the chunk's two sums
                rows, DRAM-bounce broadcast (SBUF-source broadcast DMAs
                serialize on one SBUF port, ~12us), one gpsimd mul."""
                def norm():
                    cqs = slice(cq * 512, cq * 512 + 512)
                    recip = stg.tile([2, 512], F32, tag="recip",
                                     name=f"recip_p{p}c{cq}")
                    nc.vector.reciprocal(recip[:], st["stage"][:])
                    nc.sync.dma_start(out=rscr[p, 2 * cq:2 * cq + 2],
                                      in_=recip[:])
                    bc = bcp.tile([128, 512], F32, tag="bcc",
                                  name=f"bc_p{p}c{cq}")
                    for h in (0, 1):
                        bc_src = bass.AP(
                            tensor=rscr[:].tensor,
                            offset=(p * 2 * QC + 2 * cq + h) * 512,
                            ap=[[0, 64], [1, 512]],
                        )
                        nc.gpsimd.dma_start(out=bc[64 * h:64 * h + 64, :],
                                            in_=bc_src)
                    nc.gpsimd.tensor_mul(
                        onorm[:, cqs], onorm.bitcast(F32)[:, cqs], bc[:]
                    )
                return [norm]

            def gen_wout(cq, onorms):
                """Output projection for q-tiles of chunk cq, accumulating
                all 4 pairs' onorm in PSUM."""
                def wstep(qi):
                    ysb = yp.tile([128, DM], F32, tag="y", name=f"y_{qi}")
                    for nh in range(2):
                        yps = psf.tile([128, 512], F32, tag="bank",
                                       name=f"yps_{qi}_{nh}")
                        for p in range(NPAIR):
                            nc.tensor.matmul(
                                yps[:],
                                onorms[p][:, qi * 128:(qi + 1) * 128],
                                wout_t[:, p, nh * 512:(nh + 1) * 512],
                                start=(p == 0), stop=(p == NPAIR - 1),
                            )
                        if nh == 0:
                            nc.scalar.copy(ysb[:, 0:512], yps[:])
                        else:
                            nc.vector.tensor_copy(ysb[:, 512:1024], yps[:])
                    nc.sync.dma_start(
                        out=y[qi * 128:(qi + 1) * 128, :], in_=ysb
                    )
                return [lambda qi=qi: wstep(qi)
                        for qi in range(4 * cq, 4 * cq + 4)]

            def drive_chunk(steps, fq, quota):
                """Run chunk steps, dripping up to `quota` thunks from fq
                evenly between steps."""
                n = len(steps)
                fi = 0
                for i, t in enumerate(steps):
                    t()
                    want = min(quota, len(fq) + fi) * (i + 1) // n
                    while fi < want and fq:
                        fq.popleft()()
                        fi += 1

            # ---- main schedule ------------------------------------------
            onorms = []
            cur = {}
            fq = deque()
            for t in gen_proj(0, cur):
                t()
            # k-tiles 0-3 of V_aug are needed by attention chunk 0
            # immediately; each chunk then emits the NEXT chunk's four
            # transposes as a mandatory prefix (PE filler that is
            # guaranteed to precede the PV matmuls that read the tiles —
            # leaving them in the drip queue can emit them after).
            vtr = gen_vtrans(0, cur["vt"], vaugs[0])
            for t in vtr[:4]:
                t()
            vtr_rest = vtr[4:]

            for p in range(NPAIR):
                onorm = onp.tile([128, SEQ], F32R, tag="onorm",
                                 name=f"onorm_{p}")
                onorms.append(onorm)
                chunks = gen_attention_chunks(p, cur["qt"], cur["kt"],
                                              onorm, vaugs[p % 2])
                nxt = {}
                if p + 1 < NPAIR:
                    fq.extend(gen_proj(p + 1, nxt))
                total = sum(len(s) for s, _ in chunks)
                done = 0
                for cq, (steps, st) in enumerate(chunks):
                    for t in vtr_rest[4 * cq:4 * cq + 4]:
                        t()
                    if p == NPAIR - 1 and cq >= 2:
                        fq.extend(gen_wout(cq - 2, onorms))
                    rem = total - done
                    quota = (len(fq) * len(steps) + rem - 1) // rem
                    drive_chunk(steps, fq, quota)
                    done += len(steps)
                    if cq < QC - 1:
                        fq.extendleft(
                            reversed(gen_norm_chunk(p, cq, onorm, st)))
                # flush before the next pair's transposes (the leftover
                # projection steps write vt, which vtrans reads).
                while fq:
                    fq.popleft()()
                if p + 1 < NPAIR:
                    vtr = gen_vtrans(p + 1, nxt["vt"], vaugs[(p + 1) % 2])
                    for t in vtr[:4]:
                        t()
                    vtr_rest = vtr[4:]
                else:
                    vtr_rest = []
                # the boundary chunk's normalize goes to the back of the
                # queue so its 3.3us DVE reciprocal lands behind the next
                # pair's first diagonal mask-muls and V_aug copies.
                fq.extend(gen_norm_chunk(p, QC - 1, onorm, chunks[QC - 1][1]))
                cur = nxt
            # tail: last chunk's normalize first, then wout(2) overlaps the
            # normalize chain's latency before the gated wout(3) runs.
            while fq:
                fq.popleft()()
            for t in gen_wout(2, onorms):
                t()
            for t in gen_wout(3, onorms):
                t()

    return _patch_nc(nc)


def _causal_masks():
    # masks[kp, j, f] = f >= kp + 128*j  (block-diagonal causal masks)
    m = np.zeros((128, 4, 512), np.float32)
    i = np.arange(128)[:, None]
    f = np.arange(512)[None, :]
    for j in range(4):
        m[:, j, :] = (f >= i + 128 * j).astype(np.float32)
    return m


def _prepare_in_maps(residual_stream, weight_query, weight_key, weight_value,
                     weight_out):
    bf16 = ml_dtypes.bfloat16
    r = np.asarray(residual_stream, np.float32)
    xts = []
    for b in range(B):
        xtb = r[b].T.reshape(MT, 128, QC, 512)          # [m, p, cq, q]
        xtb = xtb.transpose(1, 2, 0, 3)                 # [p, cq, m, q]
        xts.append(np.ascontiguousarray(xtb.astype(bf16)))
    masks = _causal_masks().astype(bf16)
    ident = np.eye(128, dtype=np.float32).astype(bf16)
    ones = np.ones((1, 64), bf16)
    wo_f = np.asarray(weight_out, np.float32)
    in_maps = []
    for c in range(NCORES):
        b, g = c // 2, c % 2
        w = np.empty((128, MT, 3, NPAIR * 128), np.float32)
        for t, wt in enumerate((weight_query, weight_key, weight_value)):
            wg = np.asarray(wt, np.float32)[8 * g:8 * g + 8]
            wg = wg.reshape(NPAIR, 2, MT, 128, DH)      # [p, hh, m, kp, d]
            w[:, :, t, :] = wg.transpose(3, 2, 0, 1, 4).reshape(
                128, MT, NPAIR * 128)
        wo = wo_f[512 * g:512 * (g + 1)].reshape(NPAIR, 128, DM)
        wo = np.ascontiguousarray(wo.transpose(1, 0, 2))
        in_maps.append({
            "xt": xts[b],
            "w_all": np.ascontiguousarray(w.astype(bf16)),
            "wout": wo,
            "masks": masks,
            "ident": ident,
            "ones_bf": ones,
        })
    return in_maps


def kernel(residual_stream, weight_query, weight_key, weight_value,
           weight_out, trace=False):
    from concourse.bass_utils import run_bass_kernel_spmd

    if "nc" not in _CACHE:
        _CACHE["nc"] = _build_nc()
    nc = _CACHE["nc"]

    in_maps = _prepare_in_maps(
        residual_stream, weight_query, weight_key, weight_value, weight_out
    )
    res = run_bass_kernel_spmd(
        nc, in_maps, list(range(NCORES)), trace=trace
    )
    _CACHE["last_result"] = res
    out = np.empty((B, SEQ, DM), np.float32)
    for b in range(B):
        out[b] = res.results[2 * b]["y"] + res.results[2 * b + 1]["y"]
    return out
